# revision 9
# baseline (speedup 1.0000x reference)
"""AffinityPropagate3D Trainium2 kernel.

Strategy (8 NeuronCores):
  - Spatial split along W: 320 -> 8 chunks of 40 cols, plus 2-col halos.
  - On-chip layout: partitions = H (128). All 24 propagation iterations run
    with state resident in SBUF.
  - Per 3x3x3 tap (27 of them): the per-voxel multiply f * aff runs on the
    Vector engine in fp16 (2x packed mode); the 27-tap summation runs on the
    Tensor engine as matmuls with banded 0/1 "shift" matrices (which also
    realize the H-direction stencil shift and its zero boundary), accumulating
    in PSUM fp32.
  - Affinity normalization (abs-sum / divide / degree channel) happens on
    device in a prologue; the H-shifted per-tap affinity slabs are produced
    with partition-shifted SBUF->SBUF DMA copies.
  - Per-iteration 1-col halo exchange between neighboring cores via an
    8-core AllGather (DRAM bounce), applied with per-core 0/1 mask constants
    so the SPMD program is identical on every core.
"""
import sys

sys.path.insert(0, "/opt/trn_rl_repo")

from contextlib import ExitStack

import numpy as np

import concourse.bacc as bacc
import concourse.bass as bass
import concourse.tile as tile
from concourse import mybir
from concourse.bass_utils import run_bass_kernel_spmd

B = 2          # batch
C = 26         # affinity channels
CF = 4         # feature channels
D = 16         # depth
H = 128        # height = partition dim
W = 320        # width
NCORES = 8
WC = W // NCORES   # interior cols per core (40)
WL = WC + 4        # local padded width (44): [pad, halo, 40 interior, halo, pad]
DP = D + 2         # padded depth
T_DEFAULT = 24

F16 = mybir.dt.float16
F32 = mybir.dt.float32

OFFV = (2, 0, 1)  # reference tap traversal order; delta = off - 1


def tap_info(t):
    pd = OFFV[t // 9]
    ph = OFFV[(t // 3) % 3]
    pw = OFFV[t % 3]
    return pd - 1, ph - 1, pw - 1  # (dd, dh, dw)


def _bcast_cf(ap_in, cf=CF):
    """Insert a stride-0 CF dim right after the partition dim of an AP."""
    return bass.AP(
        tensor=ap_in.tensor,
        offset=ap_in.offset,
        ap=[ap_in.ap[0], [0, cf]] + list(ap_in.ap[1:]),
    )


def build(times):
    nc = bacc.Bacc("TRN2", num_devices=NCORES)

    aff_in = nc.declare_dram_parameter("aff", [B, C, D, H, WL], F32, isOutput=False)
    f_in = nc.declare_dram_parameter("feat", [B, CF, D, H, WL], F16, isOutput=False)
    smat_in = nc.declare_dram_parameter("smat", [3, H, H], F16, isOutput=False)
    maskl_in = nc.declare_dram_parameter("maskl", [H, NCORES], F32, isOutput=False)
    maskr_in = nc.declare_dram_parameter("maskr", [H, NCORES], F32, isOutput=False)
    out_ext = nc.declare_dram_parameter("out", [B, CF, D, H, WC], F32, isOutput=True)

    # taps ordered by dh so the PE stationary matrix switches only 3x/iter
    tap_order = sorted(range(27), key=lambda t: tap_info(t)[1])

    with ExitStack() as ctx:
        tc = ctx.enter_context(tile.TileContext(nc))
        state = ctx.enter_context(tc.tile_pool(name="state", bufs=1))
        qpool = ctx.enter_context(tc.tile_pool(name="qpool", bufs=2))
        psum = ctx.enter_context(tc.tile_pool(name="psum", bufs=8, space="PSUM"))
        halo = ctx.enter_context(tc.tile_pool(name="halo", bufs=2))
        dram = ctx.enter_context(tc.tile_pool(name="dram", bufs=2, space="DRAM"))

        # ---- persistent state tiles ----
        slab = state.tile([H, B, 27, D, WL], F16)   # per-tap normalized affinity
        f0 = state.tile([H, B, CF, DP, WL], F16)
        f1 = state.tile([H, B, CF, DP, WL], F16)
        smats = state.tile([H, 3, H], F16)
        maskl = state.tile([H, NCORES], F32)
        maskr = state.tile([H, NCORES], F32)
        outstage = state.tile([H, B, CF, D, WC], F32)

        nc.vector.memset(f0, 0.0)
        nc.vector.memset(f1, 0.0)
        nc.vector.memset(slab, 0.0)
        nc.sync.dma_start(out=smats, in_=smat_in[:, :, :].transpose([1, 0, 2]))
        nc.sync.dma_start(out=maskl, in_=maskl_in[:, :])
        nc.sync.dma_start(out=maskr, in_=maskr_in[:, :])
        # initial feature (host supplies fp16, halos prefilled)
        for b_ in range(B):
            for cf_ in range(CF):
                nc.sync.dma_start(
                    out=f0[:, b_, cf_, 1 : 1 + D, :],
                    in_=f_in[b_, cf_, :, :, :].transpose([1, 0, 2]),
                )

        # ---- prologue: normalize affinity into per-tap slabs ----
        # class by dw: off=1 for dw=+/-1 (write s=w+1), off=0 for dw=0 (s=w)
        with tc.tile_pool(name="prol", bufs=3) as prol, \
             tc.tile_pool(name="prols", bufs=1) as prols:
            DH2 = D // 2
            for b in range(B):
                for dhalf in range(2):
                    stg = prols.tile([H, 27, DH2, WL], F16)  # unshifted slab chunk
                    for di in range(DH2):
                        d = dhalf * DH2 + di
                        st = prol.tile([H, C, WL], F32)
                        nc.sync.dma_start(
                            out=st, in_=aff_in[b, :, d, :, :].transpose([1, 0, 2])
                        )
                        # abs-sum over channels (innermost via permuted AP)
                        st_perm = st[:, :, :].transpose([0, 2, 1])  # [H, WL, C]
                        sabs = prol.tile([H, WL], F32, tag="sm")
                        nc.vector.tensor_reduce(
                            sabs, st_perm, axis=mybir.AxisListType.X,
                            op=mybir.AluOpType.add, apply_absolute_value=True,
                        )
                        nc.vector.tensor_scalar_max(sabs, sabs, 1.0)
                        rec = prol.tile([H, WL], F32, tag="sm")
                        nc.vector.reciprocal(rec, sabs)
                        # fold a 0.5x per-iteration rescale into the weights so
                        # the fp16 state stays in range (undone in final drain)
                        nc.vector.tensor_scalar_mul(rec, rec, 0.5)
                        ssum = prol.tile([H, WL], F32, tag="sm")
                        nc.vector.tensor_reduce(
                            ssum, st_perm, axis=mybir.AxisListType.X,
                            op=mybir.AluOpType.add,
                        )
                        # degree = 1 - ssum * rec   -> tap 26 (off=0)
                        tmp = prol.tile([H, WL], F32, tag="sm")
                        nc.vector.tensor_tensor(
                            tmp, ssum, rec, op=mybir.AluOpType.mult
                        )
                        nc.vector.tensor_scalar(
                            out=stg[:, 26, di, :], in0=tmp,
                            scalar1=-1.0, scalar2=0.5,
                            op0=mybir.AluOpType.mult, op1=mybir.AluOpType.add,
                        )
                        # normalized channels -> staging, 3 classes by c%3
                        # c%3==0 -> dw=+1 (off 1), c%3==1 -> dw=-1 (off 1),
                        # c%3==2 -> dw=0 (off 0)
                        for cm, off in ((0, 1), (1, 1), (2, 0)):
                            ntap = 9 if cm < 2 else 8
                            wn = WL - off  # writable cols
                            in0 = bass.AP(
                                tensor=st.tensor, offset=st[:, cm, 0].offset,
                                ap=[st[:, :, :].ap[0], [3 * WL, ntap], [1, wn]],
                            )
                            in1 = bass.AP(
                                tensor=rec.tensor, offset=rec[:, 0].offset,
                                ap=[rec[:, :].ap[0], [0, ntap], [1, wn]],
                            )
                            outap = bass.AP(
                                tensor=stg.tensor,
                                offset=stg[:, cm, di, off].offset,
                                ap=[stg[:, :, :, :].ap[0], [3 * DH2 * WL, ntap], [1, wn]],
                            )
                            nc.vector.tensor_tensor(
                                out=outap, in0=in0, in1=in1,
                                op=mybir.AluOpType.mult,
                            )
                    # shifted copies staging -> final slab (h-shift by -dh)
                    # dh=+1: taps (t//3)%3==0 -> slab[k] = stg[k-1]
                    # dh=-1: taps (t//3)%3==1 -> slab[k] = stg[k+1]
                    # dh= 0: taps (t//3)%3==2 (+ tap 26 handled within set)
                    for tm, dh in ((0, 1), (1, -1), (2, 0)):
                        tset = [t for t in range(27) if (t // 3) % 3 == tm]
                        # contiguity: {3g, 3g+1, 3g+2 pattern}: taps with
                        # (t//3)%3==tm are 3 groups of 3 consecutive t
                        assert tset == [
                            9 * g + 3 * tm + j for g in range(3) for j in range(3)
                        ]
                        if dh == 1:
                            ksrc, kdst, np_ = slice(0, H - 1), slice(1, H), H - 1
                        elif dh == -1:
                            ksrc, kdst, np_ = slice(1, H), slice(0, H - 1), H - 1
                        else:
                            ksrc, kdst, np_ = slice(0, H), slice(0, H), H
                        for g in range(3):
                            src = bass.AP(
                                tensor=stg.tensor,
                                offset=stg[ksrc, 9 * g + 3 * tm, 0, 0].offset,
                                ap=[[stg[:, :, :, :].ap[0][0], np_],
                                    [DH2 * WL, 3], [1, DH2 * WL]],
                            )
                            dst = bass.AP(
                                tensor=slab.tensor,
                                offset=slab[kdst, b, 9 * g + 3 * tm,
                                            dhalf * DH2, 0].offset,
                                ap=[[slab[:, :, :, :, :].ap[0][0], np_],
                                    [D * WL, 3], [WL, DH2], [1, WL]],
                            )
                            nc.sync.dma_start(out=dst, in_=src)

        # ---- main propagation loop ----
        fc, fn = f0, f1
        for it in range(times):
            last = it == times - 1
            for b in range(B):
                pstiles = [psum.tile([H, D // 2, WC], F32, name=f"ps{b}_{i}_{it}", tag="ps") for i in range(8)]
                for ti, t in enumerate(tap_order):
                    dd, dh, dw = tap_info(t)
                    y0 = 0 if dw == -1 else 2
                    q = qpool.tile([H, CF, D, WL], F16, tag="q")
                    nc.vector.tensor_tensor(
                        out=q[:, :, :, y0 : y0 + 42],
                        in0=fc[:, b, :, 1 + dd : 1 + dd + D, y0 : y0 + 42],
                        in1=_bcast_cf(slab[:, b, t, :, 2:44]),
                        op=mybir.AluOpType.mult,
                    )
                    lhsT = smats[:, dh + 1, :]
                    for cf in range(CF):
                        for k in range(2):
                            nc.tensor.matmul(
                                pstiles[cf * 2 + k][:, :, :],
                                lhsT=lhsT,
                                rhs=q[:, cf, k * 8 : k * 8 + 8, 2 + dw : 42 + dw],
                                start=(ti == 0),
                                stop=(ti == 26),
                            )
                for cf in range(CF):
                    for k in range(2):
                        ps = pstiles[cf * 2 + k]
                        if last:
                            nc.scalar.mul(
                                out=outstage[:, b, cf, k * 8 : k * 8 + 8, :],
                                in_=ps[:, :, :],
                                mul=float(2 ** times),
                            )
                        else:
                            nc.scalar.copy(
                                out=fn[:, b, cf, 1 + k * 8 : 9 + k * 8, 2:42],
                                in_=ps[:, :, :],
                            )
            if not last:
                # halo exchange on fn: send cols 2 and 41, receive into 42 / 1
                stg = halo.tile([H, 2, B, CF, D], F16, tag="hstg")
                nc.vector.tensor_copy(out=stg[:, 0], in_=fn[:, :, :, 1 : 1 + D, 2])
                nc.vector.tensor_copy(out=stg[:, 1], in_=fn[:, :, :, 1 : 1 + D, 41])
                ib = dram.tile([H, 2 * B * CF * D], F16, tag="ib")
                ob = dram.tile([NCORES, H, 2 * B * CF * D], F16, tag="ob")
                nc.gpsimd.dma_start(out=ib, in_=stg[:, :, :, :, :])
                nc.gpsimd.collective_compute(
                    "AllGather",
                    mybir.AluOpType.bypass,
                    replica_groups=[list(range(NCORES))],
                    ins=[ib[:].opt()],
                    outs=[ob[:].opt()],
                )
                gsb = halo.tile([H, NCORES, 2 * B * CF * D], F16, tag="gsb")
                nc.gpsimd.dma_start(out=gsb, in_=ob[:].transpose([1, 0, 2]))
                # my col 42 <- slot j's "col 2" package  (mask_r selects j=c+1)
                # my col 1  <- slot j's "col 41" package (mask_l selects j=c-1)
                for dstcol, pkg, mask in ((42, 0, maskr), (1, 1, maskl)):
                    dst = fn[:, :, :, 1 : 1 + D, dstcol]
                    for j in range(NCORES):
                        src = bass.AP(
                            tensor=gsb.tensor,
                            offset=gsb[:, j, pkg * B * CF * D].offset,
                            ap=[gsb[:, :, :].ap[0], [CF * D, B], [D, CF], [1, D]],
                        )
                        if j == 0:
                            nc.vector.tensor_scalar(
                                out=dst, in0=src, scalar1=mask[:, 0:1],
                                scalar2=None, op0=mybir.AluOpType.mult,
                            )
                        else:
                            nc.vector.scalar_tensor_tensor(
                                out=dst, in0=src, scalar=mask[:, j : j + 1],
                                in1=dst, op0=mybir.AluOpType.mult,
                                op1=mybir.AluOpType.add,
                            )
            fc, fn = fn, fc
        nc.sync.dma_start(
            out=out_ext[:, :, :, :, :].transpose([3, 0, 1, 2, 4]),
            in_=outstage,
        )

    nc.compile()
    return nc


_cache = {}


def _get_nc(times):
    if times not in _cache:
        _cache[times] = build(times)
    return _cache[times]


def make_in_maps(affinity, feature):
    affinity = np.asarray(affinity, dtype=np.float32)
    feature = np.asarray(feature, dtype=np.float32)
    smat = np.zeros((3, H, H), np.float16)
    for i, dh in enumerate((-1, 0, 1)):
        for m in range(H):
            k = m + dh
            if 0 <= k < H:
                smat[i, k, m] = 1.0

    in_maps = []
    for c in range(NCORES):
        lo = c * WC - 2
        hi = c * WC + WC + 2
        plo, phi = max(0, -lo), max(0, hi - W)
        aff_sl = np.pad(
            affinity[:, :, :, :, max(lo, 0) : min(hi, W)],
            ((0, 0), (0, 0), (0, 0), (0, 0), (plo, phi)),
        )
        f_sl = np.pad(
            feature[:, :, :, :, max(lo, 0) : min(hi, W)],
            ((0, 0), (0, 0), (0, 0), (0, 0), (plo, phi)),
        ).astype(np.float16)
        ml = np.zeros((H, NCORES), np.float32)
        mr = np.zeros((H, NCORES), np.float32)
        if c > 0:
            ml[:, c - 1] = 1.0
        if c < NCORES - 1:
            mr[:, c + 1] = 1.0
        in_maps.append(
            {"aff": aff_sl, "feat": f_sl, "smat": smat, "maskl": ml, "maskr": mr}
        )
    return in_maps


def kernel(affinity, feature, times=T_DEFAULT):
    times = int(times)
    nc = _get_nc(times)
    in_maps = make_in_maps(affinity, feature)
    res = run_bass_kernel_spmd(nc, in_maps, core_ids=list(range(NCORES)))
    out = np.concatenate([res.results[c]["out"] for c in range(NCORES)], axis=-1)
    return out


# revision 12
# speedup vs baseline: 945.0232x; 945.0232x over previous
"""AffinityPropagate3D Trainium2 kernel.

Strategy (8 NeuronCores):
  - Spatial split along W: 320 -> 8 chunks of 40 cols, plus 2-col halos.
  - On-chip layout: partitions = H (128). All 24 propagation iterations run
    with state resident in SBUF.
  - Per 3x3x3 tap (27 of them): the per-voxel multiply f * aff runs on the
    Vector engine in fp16 (2x packed mode); the 27-tap summation runs on the
    Tensor engine as matmuls with banded 0/1 "shift" matrices (which also
    realize the H-direction stencil shift and its zero boundary), accumulating
    in PSUM fp32.
  - Affinity normalization (abs-sum / divide / degree channel) happens on
    device in a prologue; the H-shifted per-tap affinity slabs are produced
    with partition-shifted SBUF->SBUF DMA copies.
  - Per-iteration 1-col halo exchange between neighboring cores via an
    8-core AllGather (DRAM bounce), applied with per-core 0/1 mask constants
    so the SPMD program is identical on every core.
"""
import sys

sys.path.insert(0, "/opt/trn_rl_repo")

from contextlib import ExitStack

import numpy as np

import concourse.bacc as bacc
import concourse.bass as bass
import concourse.tile as tile
from concourse import mybir

B = 2          # batch
C = 26         # affinity channels
CF = 4         # feature channels
D = 16         # depth
H = 128        # height = partition dim
W = 320        # width
NCORES = 8
WC = W // NCORES   # interior cols per core (40)
WL = WC + 4        # local padded width (44): [pad, halo, 40 interior, halo, pad]
DP = D + 2         # padded depth
T_DEFAULT = 24

F16 = mybir.dt.float16
F32 = mybir.dt.float32

OFFV = (2, 0, 1)  # reference tap traversal order; delta = off - 1


def tap_info(t):
    pd = OFFV[t // 9]
    ph = OFFV[(t // 3) % 3]
    pw = OFFV[t % 3]
    return pd - 1, ph - 1, pw - 1  # (dd, dh, dw)


def _bcast_cf(ap_in, cf=CF):
    """Insert a stride-0 CF dim right after the partition dim of an AP."""
    return bass.AP(
        tensor=ap_in.tensor,
        offset=ap_in.offset,
        ap=[ap_in.ap[0], [0, cf]] + list(ap_in.ap[1:]),
    )


def build(times):
    nc = bacc.Bacc("TRN2", num_devices=NCORES)

    aff_in = nc.declare_dram_parameter("aff", [B, C, D, H, WL], F32, isOutput=False)
    f_in = nc.declare_dram_parameter("feat", [B, CF, D, H, WL], F16, isOutput=False)
    smat_in = nc.declare_dram_parameter("smat", [3, H, H], F16, isOutput=False)
    maskl_in = nc.declare_dram_parameter("maskl", [H, NCORES], F32, isOutput=False)
    maskr_in = nc.declare_dram_parameter("maskr", [H, NCORES], F32, isOutput=False)
    out_ext = nc.declare_dram_parameter("out", [B, CF, D, H, WC], F32, isOutput=True)

    # taps ordered by dh so the PE stationary matrix switches only 3x/iter
    tap_order = sorted(range(27), key=lambda t: tap_info(t)[1])

    with ExitStack() as ctx:
        tc = ctx.enter_context(tile.TileContext(nc))
        state = ctx.enter_context(tc.tile_pool(name="state", bufs=1))
        qpool = ctx.enter_context(tc.tile_pool(name="qpool", bufs=3))
        psum = ctx.enter_context(tc.tile_pool(name="psum", bufs=8, space="PSUM"))
        halo = ctx.enter_context(tc.tile_pool(name="halo", bufs=2))
        dram = ctx.enter_context(tc.tile_pool(name="dram", bufs=2, space="DRAM"))

        # ---- persistent state tiles ----
        slab = state.tile([H, B, 27, D, WL], F16)   # per-tap normalized affinity
        f0 = state.tile([H, B, CF, DP, WL], F16)
        f1 = state.tile([H, B, CF, DP, WL], F16)
        smats = state.tile([H, 3, H], F16)
        maskl = state.tile([H, NCORES], F32)
        maskr = state.tile([H, NCORES], F32)
        outstage = state.tile([H, B, CF, D, WC], F32)

        nc.vector.memset(f0, 0.0)
        nc.vector.memset(f1, 0.0)
        nc.vector.memset(slab, 0.0)
        nc.sync.dma_start(out=smats, in_=smat_in[:, :, :].transpose([1, 0, 2]))
        nc.sync.dma_start(out=maskl, in_=maskl_in[:, :])
        nc.sync.dma_start(out=maskr, in_=maskr_in[:, :])
        # initial feature (host supplies fp16, halos prefilled)
        for b_ in range(B):
            for cf_ in range(CF):
                nc.sync.dma_start(
                    out=f0[:, b_, cf_, 1 : 1 + D, :],
                    in_=f_in[b_, cf_, :, :, :].transpose([1, 0, 2]),
                )

        # ---- prologue: normalize affinity into per-tap slabs ----
        # class by dw: off=1 for dw=+/-1 (write s=w+1), off=0 for dw=0 (s=w)
        with tc.tile_pool(name="prol", bufs=3) as prol, \
             tc.tile_pool(name="prols", bufs=1) as prols:
            DH2 = D // 2
            for b in range(B):
                for dhalf in range(2):
                    stg = prols.tile([H, 27, DH2, WL], F16)  # unshifted slab chunk
                    for di in range(DH2):
                        d = dhalf * DH2 + di
                        st = prol.tile([H, C, WL], F32)
                        nc.sync.dma_start(
                            out=st, in_=aff_in[b, :, d, :, :].transpose([1, 0, 2])
                        )
                        # abs-sum over channels (innermost via permuted AP)
                        st_perm = st[:, :, :].transpose([0, 2, 1])  # [H, WL, C]
                        sabs = prol.tile([H, WL], F32, tag="sm")
                        nc.vector.tensor_reduce(
                            sabs, st_perm, axis=mybir.AxisListType.X,
                            op=mybir.AluOpType.add, apply_absolute_value=True,
                        )
                        nc.vector.tensor_scalar_max(sabs, sabs, 1.0)
                        rec = prol.tile([H, WL], F32, tag="sm")
                        nc.vector.reciprocal(rec, sabs)
                        # fold a 0.5x per-iteration rescale into the weights so
                        # the fp16 state stays in range (undone in final drain)
                        nc.vector.tensor_scalar_mul(rec, rec, 0.5)
                        ssum = prol.tile([H, WL], F32, tag="sm")
                        nc.vector.tensor_reduce(
                            ssum, st_perm, axis=mybir.AxisListType.X,
                            op=mybir.AluOpType.add,
                        )
                        # degree = 1 - ssum * rec   -> tap 26 (off=0)
                        tmp = prol.tile([H, WL], F32, tag="sm")
                        nc.vector.tensor_tensor(
                            tmp, ssum, rec, op=mybir.AluOpType.mult
                        )
                        nc.vector.tensor_scalar(
                            out=stg[:, 26, di, :], in0=tmp,
                            scalar1=-1.0, scalar2=0.5,
                            op0=mybir.AluOpType.mult, op1=mybir.AluOpType.add,
                        )
                        # normalized channels -> staging, 3 classes by c%3
                        # c%3==0 -> dw=+1 (off 1), c%3==1 -> dw=-1 (off 1),
                        # c%3==2 -> dw=0 (off 0)
                        for cm, off in ((0, 1), (1, 1), (2, 0)):
                            ntap = 9 if cm < 2 else 8
                            wn = WL - off  # writable cols
                            in0 = bass.AP(
                                tensor=st.tensor, offset=st[:, cm, 0].offset,
                                ap=[st[:, :, :].ap[0], [3 * WL, ntap], [1, wn]],
                            )
                            in1 = bass.AP(
                                tensor=rec.tensor, offset=rec[:, 0].offset,
                                ap=[rec[:, :].ap[0], [0, ntap], [1, wn]],
                            )
                            outap = bass.AP(
                                tensor=stg.tensor,
                                offset=stg[:, cm, di, off].offset,
                                ap=[stg[:, :, :, :].ap[0], [3 * DH2 * WL, ntap], [1, wn]],
                            )
                            nc.vector.tensor_tensor(
                                out=outap, in0=in0, in1=in1,
                                op=mybir.AluOpType.mult,
                            )
                    # shifted copies staging -> final slab (h-shift by -dh)
                    # dh=+1: taps (t//3)%3==0 -> slab[k] = stg[k-1]
                    # dh=-1: taps (t//3)%3==1 -> slab[k] = stg[k+1]
                    # dh= 0: taps (t//3)%3==2 (+ tap 26 handled within set)
                    for tm, dh in ((0, 1), (1, -1), (2, 0)):
                        tset = [t for t in range(27) if (t // 3) % 3 == tm]
                        # contiguity: {3g, 3g+1, 3g+2 pattern}: taps with
                        # (t//3)%3==tm are 3 groups of 3 consecutive t
                        assert tset == [
                            9 * g + 3 * tm + j for g in range(3) for j in range(3)
                        ]
                        if dh == 1:
                            ksrc, kdst, np_ = slice(0, H - 1), slice(1, H), H - 1
                        elif dh == -1:
                            ksrc, kdst, np_ = slice(1, H), slice(0, H - 1), H - 1
                        else:
                            ksrc, kdst, np_ = slice(0, H), slice(0, H), H
                        for g in range(3):
                            src = bass.AP(
                                tensor=stg.tensor,
                                offset=stg[ksrc, 9 * g + 3 * tm, 0, 0].offset,
                                ap=[[stg[:, :, :, :].ap[0][0], np_],
                                    [DH2 * WL, 3], [1, DH2 * WL]],
                            )
                            dst = bass.AP(
                                tensor=slab.tensor,
                                offset=slab[kdst, b, 9 * g + 3 * tm,
                                            dhalf * DH2, 0].offset,
                                ap=[[slab[:, :, :, :, :].ap[0][0], np_],
                                    [D * WL, 3], [WL, DH2], [1, WL]],
                            )
                            nc.sync.dma_start(out=dst, in_=src)

        # ---- main propagation loop ----
        fc, fn = f0, f1
        for it in range(times):
            last = it == times - 1
            for b in range(B):
                pstiles = [psum.tile([H, D // 2, WC], F32, name=f"ps{b}_{i}_{it}", tag="ps") for i in range(8)]
                for ti, t in enumerate(tap_order):
                    dd, dh, dw = tap_info(t)
                    y0 = 0 if dw == -1 else 2
                    wn = 40 if dw == 0 else 42
                    q = qpool.tile([H, CF, D, WL], F16, tag="q")
                    nc.vector.tensor_tensor(
                        out=q[:, :, :, y0 : y0 + wn],
                        in0=fc[:, b, :, 1 + dd : 1 + dd + D, y0 : y0 + wn],
                        in1=_bcast_cf(slab[:, b, t, :, 2 : 2 + wn]),
                        op=mybir.AluOpType.mult,
                    )
                    lhsT = smats[:, dh + 1, :]
                    for cf in range(CF):
                        for k in range(2):
                            nc.tensor.matmul(
                                pstiles[cf * 2 + k][:, :, :],
                                lhsT=lhsT,
                                rhs=q[:, cf, k * 8 : k * 8 + 8, 2 + dw : 42 + dw],
                                start=(ti == 0),
                                stop=(ti == 26),
                            )
                for cf in range(CF):
                    for k in range(2):
                        ps = pstiles[cf * 2 + k]
                        if last:
                            nc.scalar.mul(
                                out=outstage[:, b, cf, k * 8 : k * 8 + 8, :],
                                in_=ps[:, :, :],
                                mul=float(2 ** times),
                            )
                        else:
                            nc.scalar.copy(
                                out=fn[:, b, cf, 1 + k * 8 : 9 + k * 8, 2:42],
                                in_=ps[:, :, :],
                            )
            import os as _os
            if _os.environ.get("KERNEL_NO_HALO"):
                fc, fn = fn, fc
                continue
            if not last:
                # halo exchange on fn: send cols 2 and 41, receive into 42 / 1
                stg = halo.tile([H, 2, B, CF, D], F16, tag="hstg")
                nc.vector.tensor_copy(out=stg[:, 0], in_=fn[:, :, :, 1 : 1 + D, 2])
                nc.vector.tensor_copy(out=stg[:, 1], in_=fn[:, :, :, 1 : 1 + D, 41])
                ib = dram.tile([H, 2 * B * CF * D], F16, tag="ib")
                ob = dram.tile([NCORES, H, 2 * B * CF * D], F16, tag="ob")
                nc.gpsimd.dma_start(out=ib, in_=stg[:, :, :, :, :])
                nc.gpsimd.collective_compute(
                    "AllGather",
                    mybir.AluOpType.bypass,
                    replica_groups=[list(range(NCORES))],
                    ins=[ib[:].opt()],
                    outs=[ob[:].opt()],
                )
                gsb = halo.tile([H, NCORES, 2 * B * CF * D], F16, tag="gsb")
                nc.gpsimd.dma_start(out=gsb, in_=ob[:].transpose([1, 0, 2]))
                # my col 42 <- slot j's "col 2" package  (mask_r selects j=c+1)
                # my col 1  <- slot j's "col 41" package (mask_l selects j=c-1)
                for dstcol, pkg, mask in ((42, 0, maskr), (1, 1, maskl)):
                    dst = fn[:, :, :, 1 : 1 + D, dstcol]
                    for j in range(NCORES):
                        src = bass.AP(
                            tensor=gsb.tensor,
                            offset=gsb[:, j, pkg * B * CF * D].offset,
                            ap=[gsb[:, :, :].ap[0], [CF * D, B], [D, CF], [1, D]],
                        )
                        if j == 0:
                            nc.vector.tensor_scalar(
                                out=dst, in0=src, scalar1=mask[:, 0:1],
                                scalar2=None, op0=mybir.AluOpType.mult,
                            )
                        else:
                            nc.vector.scalar_tensor_tensor(
                                out=dst, in0=src, scalar=mask[:, j : j + 1],
                                in1=dst, op0=mybir.AluOpType.mult,
                                op1=mybir.AluOpType.add,
                            )
            fc, fn = fn, fc
        nc.sync.dma_start(
            out=out_ext[:, :, :, :, :].transpose([3, 0, 1, 2, 4]),
            in_=outstage,
        )

    nc.compile()
    return nc


_cache = {}


def _get_nc(times):
    if times not in _cache:
        _cache[times] = build(times)
    return _cache[times]


class _Runner:
    """Caches the jitted 8-core PJRT executable for a built Bass module so
    repeat kernel() calls skip jax retracing (run_bass_kernel_spmd rebuilds
    the jit closure per call)."""

    def __init__(self, nc):
        import jax
        from jax.sharding import Mesh, PartitionSpec
        from jax.experimental.shard_map import shard_map
        from concourse.bass2jax import (
            _bass_exec_p,
            install_neuronx_cc_hook,
            partition_id_tensor,
        )

        install_neuronx_cc_hook()
        self.jax = jax
        pname = nc.partition_id_tensor.name if nc.partition_id_tensor else None
        in_names, out_names, out_avals, zero_outs = [], [], [], []
        for alloc in nc.m.functions[0].allocations:
            if not isinstance(alloc, mybir.MemoryLocationSet):
                continue
            name = alloc.memorylocations[0].name
            if alloc.kind == "ExternalInput":
                if name != pname:
                    in_names.append(name)
            elif alloc.kind == "ExternalOutput":
                out_names.append(name)
                shape = tuple(alloc.tensor_shape)
                dtype = mybir.dt.np(alloc.dtype)
                out_avals.append(jax.core.ShapedArray(shape, dtype))
                zero_outs.append(np.zeros(shape, dtype))
        self.in_names, self.out_names = in_names, out_names
        self.zero_outs = zero_outs
        n_params, n_outs = len(in_names), len(out_names)
        full_in = in_names + out_names + ([pname] if pname else [])

        def _body(*args):
            operands = list(args)
            if pname is not None:
                operands.append(partition_id_tensor())
            return tuple(
                _bass_exec_p.bind(
                    *operands,
                    out_avals=tuple(out_avals),
                    in_names=tuple(full_in),
                    out_names=tuple(out_names),
                    lowering_input_output_aliases=(),
                    sim_require_finite=True,
                    sim_require_nnan=True,
                    nc=nc,
                )
            )

        devices = jax.devices()[:NCORES]
        mesh = Mesh(np.asarray(devices), ("core",))
        self.fn = jax.jit(
            shard_map(
                _body,
                mesh=mesh,
                in_specs=(PartitionSpec("core"),) * (n_params + n_outs),
                out_specs=(PartitionSpec("core"),) * n_outs,
                check_rep=False,
            ),
            donate_argnums=tuple(range(n_params, n_params + n_outs)),
            keep_unused=True,
        )

    def run(self, in_maps):
        n = len(in_maps)
        concat_in = [
            np.concatenate([np.asarray(m[name]) for m in in_maps], axis=0)
            for name in self.in_names
        ]
        big_zeros = [np.concatenate([z] * n, axis=0) for z in self.zero_outs]
        outs = self.fn(*concat_in, *big_zeros)
        outs = [np.asarray(o) for o in outs]
        res = []
        for c in range(n):
            m = {}
            for i, name in enumerate(self.out_names):
                sz = self.zero_outs[i].shape[0]
                m[name] = outs[i][c * sz : (c + 1) * sz]
            res.append(m)
        return res


_runner_cache = {}


def _get_runner(times):
    if times not in _runner_cache:
        _runner_cache[times] = _Runner(_get_nc(times))
    return _runner_cache[times]


def make_in_maps(affinity, feature):
    affinity = np.asarray(affinity, dtype=np.float32)
    feature = np.asarray(feature, dtype=np.float32)
    smat = np.zeros((3, H, H), np.float16)
    for i, dh in enumerate((-1, 0, 1)):
        for m in range(H):
            k = m + dh
            if 0 <= k < H:
                smat[i, k, m] = 1.0

    in_maps = []
    for c in range(NCORES):
        lo = c * WC - 2
        hi = c * WC + WC + 2
        plo, phi = max(0, -lo), max(0, hi - W)
        aff_sl = np.pad(
            affinity[:, :, :, :, max(lo, 0) : min(hi, W)],
            ((0, 0), (0, 0), (0, 0), (0, 0), (plo, phi)),
        )
        f_sl = np.pad(
            feature[:, :, :, :, max(lo, 0) : min(hi, W)],
            ((0, 0), (0, 0), (0, 0), (0, 0), (plo, phi)),
        ).astype(np.float16)
        ml = np.zeros((H, NCORES), np.float32)
        mr = np.zeros((H, NCORES), np.float32)
        if c > 0:
            ml[:, c - 1] = 1.0
        if c < NCORES - 1:
            mr[:, c + 1] = 1.0
        in_maps.append(
            {"aff": aff_sl, "feat": f_sl, "smat": smat, "maskl": ml, "maskr": mr}
        )
    return in_maps


def kernel(affinity, feature, times=T_DEFAULT):
    times = int(times)
    runner = _get_runner(times)
    in_maps = make_in_maps(affinity, feature)
    results = runner.run(in_maps)
    out = np.concatenate([results[c]["out"] for c in range(NCORES)], axis=-1)
    return out
